# revision 1
# baseline (speedup 1.0000x reference)
"""Trainium2 Bass kernel for the stacked spiking-LSTM (SLSTM) network.

Problem: x[T=100, B=4096, C=14] -> two snntorch-style SLSTM layers (H=128,
reset_mechanism='subtract', threshold inputs thr1/thr2) -> mean over time of
layer-2 membrane potential -> linear head [B, 7].

Key mathematical property (exploited by the fast path, with a runtime guard):
the spike nonlinearity fires iff mem > thr, and mem = sigmoid(o)*tanh(c)
- reset*thr where |sigmoid(o)*tanh(c)| <= 1 in exact *and* fp32 arithmetic
(both factors saturate at 1.0; a product of two numbers <= 1 rounds to <= 1).
Hence whenever thr1 >= 1.0, layer 1 can never emit a spike, for ANY x and any
weights (even NaN/Inf inputs: NaN > thr is False).  Layer 2 then receives
identically-zero input, so its recurrence is independent of both x and the
batch index, and every output row equals

    out_row = (1/T * sum_t mem2_t) @ Wfc.T + bfc

where mem2_t follows the zero-input LSTM recurrence.  When additionally
thr2 >= 1.0 (the benchmark case) the same saturation argument kills layer-2's
resets, and the HW fast path computes the recurrence on the NeuronCores
(batch-1 column layout, one fused tanh per step via sigmoid(x) =
0.5*tanh(x/2)+0.5 with weight pre-scaling, fp16 matmul operands for Fast
Weight Load, fp32 everywhere else) and broadcasts the row on-device into each
core's batch shard.  thr2 < 1.0 falls back to an exact fp32 CPU layer-2 path
(reset decisions can be margin-critical there); thr1 < 1.0 falls back to a
full-fidelity CPU implementation.  Neither fallback triggers for this
problem's inputs.

Sharding: pure data parallel over batch, B/8 = 512 rows per core; each core
produces its own [7, 512] output shard (identical contents by the theorem
above), gathered and transposed on the host into [B, 7].
"""

import numpy as np

H = 128          # hidden size
NCO = 7          # number of classes
N_CORES = 8

# gate columns [i, f, o, g]; torch gate order in the 4H dim is i,f,g,o
_GATE_OFFS = (0, H, 3 * H, 2 * H)
_GATE_SCALES = (0.5, 0.5, 0.5, 1.0)   # 0.5 for sigmoid-via-tanh gates

_prog_cache: dict = {}


def _build_fast_program(T: int, b_shard: int):
    """Bass/Tile program: zero-input layer-2 SLSTM recurrence at batch 1,
    time-mean, linear head, and on-device broadcast to [NCO, b_shard].

    The recurrent matmul operands (Whh2 chunks and the mem state) are fp16 —
    this engages the PE Fast Weight Load path (4x faster LDWEIGHTS, which sits
    on the serial critical chain) and costs only ~3e-5 relative error on the
    final output (measured; weights are ~1e-1 scale so fp16 keeps ~11 bits).
    Everything else (bias, syn/msum accumulation, activations, head) is fp32.

    Only built for the reset-free regime (thr2 >= 1.0, or thr2 NaN): there
    the layer-2 reset is provably always zero (same saturation argument as
    layer 1: |sigmoid*tanh| <= 1, and NaN > thr is False), so the reset terms
    are dropped from the program entirely and thr2 never enters it."""
    import concourse.bass as bass
    import concourse.bacc as bacc
    import concourse.tile as tile
    import concourse.mybir as mybir

    dt = mybir.dt.float32
    dth = mybir.dt.float16
    Alu = mybir.AluOpType
    Act = mybir.ActivationFunctionType

    # Bacc (not raw Bass): its compile() runs generate_event_semaphores,
    # which splits multi-semaphore waits down to the HW's 1-wait/instruction.
    nc = bacc.Bacc(
        "TRN2", target_bir_lowering=False, debug=False, num_devices=N_CORES
    )
    # fp32 constants travel in one packed tensor (single DMA):
    # [:, 0:NCO]=wfc, [0:NCO, NCO]=bfc, [0:4, 8:8+H]=bT, [0:4, 8+H:12+H]=id4
    PW = 12 + H
    w_d = nc.dram_tensor("w", [H, 4 * H], dth, kind="ExternalInput")
    p_d = nc.dram_tensor("p", [H, PW], dt, kind="ExternalInput")
    out_d = nc.dram_tensor("out", [NCO, b_shard], dt, kind="ExternalOutput")

    with tile.TileContext(nc) as tc:
        with (
            tc.tile_pool(name="const", bufs=1) as cpool,
            tc.tile_pool(name="state", bufs=1) as spool,
            tc.tile_pool(name="work", bufs=4) as wpool,
            tc.tile_pool(name="psum", bufs=4, space=bass.MemorySpace.PSUM) as ppool,
        ):
            # Each DMA fans out over up to 8 HW queue semaphores, and compute
            # instructions have a small sync-wait budget — so every input is
            # staged through a DVE copy whose *only* dependency is its own
            # DMA; all downstream consumers then wait on the DVE semaphore.
            w_stage = cpool.tile([H, 4 * H], dth, tag="w_stage")
            p_stage = cpool.tile([H, PW], dt, tag="p_stage")
            w_sb = cpool.tile([H, 4 * H], dth, tag="w")
            p_sb = cpool.tile([H, PW], dt, tag="p")
            zsrc = cpool.tile([NCO, b_shard], dt, tag="zsrc")
            nc.sync.dma_start(w_stage[:], w_d[:])
            nc.sync.dma_start(p_stage[:], p_d[:])
            nc.vector.tensor_copy(w_sb[:], w_stage[:])
            nc.vector.tensor_copy(p_sb[:], p_stage[:])
            nc.vector.memset(zsrc[:], 0.0)
            wfc_sb = p_sb[:, 0:NCO]
            bfc_sb = p_sb[0:NCO, NCO : NCO + 1]
            bt_sb = p_sb[0:4, 8 : 8 + H]
            id4_sb = p_sb[0:4, 8 + H : 12 + H]

            syn = spool.tile([H, 1], dt, tag="syn")
            msum = spool.tile([H, 1], dt, tag="msum")
            nc.vector.memset(syn[:], 0.0)
            nc.vector.memset(msum[:], 0.0)
            mem_h = wpool.tile([H, 1], dth, tag="memh")
            nc.vector.memset(mem_h[:], 0.0)

            for _t in range(T):
                # gates (pre-scaled): ps[:, c] = b_c + w_c.T @ mem.
                # The bias lands in PSUM via a K=4 identity matmul
                # (ps[j,c] = sum_k bT[k,j]*I[k,c]) that has no dependency on
                # mem, so it runs off the critical chain.
                ps = ppool.tile([H, 4], dt, tag="ps")
                nc.tensor.matmul(
                    ps[:], bt_sb[:], id4_sb[:], start=True, stop=False,
                    skip_group_check=True,
                )
                for c in range(4):
                    nc.tensor.matmul(
                        ps[:, c : c + 1],
                        w_sb[:, c * H : (c + 1) * H],
                        mem_h[:],
                        start=False,
                        stop=(c == 3),
                        skip_group_check=True,
                    )
                # one tanh for all four gates; sigmoid(x) = 0.5*tanh(x/2)+0.5
                t4 = wpool.tile([H, 4], dt, tag="t4")
                nc.scalar.activation(t4[:], ps[:], Act.Tanh)
                # syn' = sigmoid(f)*syn + sigmoid(i)*tanh(g), factored as
                # 0.5*((t_f+1)*syn + (t_i+1)*t_g) so the two products are
                # mutually independent (both need only t4) and pipeline in the
                # DVE FIFO with no intra-chain wait (modeled 101.3us -> 85.6us)
                w = wpool.tile([H, 1], dt, tag="w2x")
                nc.vector.scalar_tensor_tensor(
                    w[:], t4[:, 0:1], 1.0, t4[:, 3:4], Alu.add, Alu.mult
                )
                z = wpool.tile([H, 1], dt, tag="z2x")
                nc.vector.scalar_tensor_tensor(
                    z[:], t4[:, 1:2], 1.0, syn[:], Alu.add, Alu.mult
                )
                # s = sigmoid(o), for the mem update and time-sum
                s = wpool.tile([H, 1], dt, tag="s")
                nc.vector.tensor_scalar(s[:], t4[:, 2:3], 0.5, 0.5, Alu.mult, Alu.add)
                nc.vector.tensor_scalar(syn[:], z[:], w[:, 0:1], 0.5, Alu.add, Alu.mult)
                tc2 = wpool.tile([H, 1], dt, tag="tc2")
                nc.scalar.activation(tc2[:], syn[:], Act.Tanh)
                # mem = sigmoid(o)*tanh(syn); reset provably always 0.
                # The fp16 copy feeding the next step's matmuls is computed on
                # ACT (Identity, per-partition scale) right after the tanh —
                # this drops the last DVE visit from the serial chain
                # (PE->ACT->DVE->ACT->PE, one less ~140ns boundary per step;
                # modeled 116.4us -> 101.3us).  The fp32 time-sum accumulates
                # off-path on DVE in one fused op.
                mem_h = wpool.tile([H, 1], dth, tag="memh")
                nc.scalar.activation(mem_h[:], tc2[:], Act.Identity, scale=s[:, 0:1])
                nc.vector.scalar_tensor_tensor(
                    msum[:], tc2[:], s[:, 0:1], msum[:], Alu.mult, Alu.add
                )

            # head: out_col = (Wfc/T).T.T @ msum + bfc  (1/T folded into wfc)
            psf = ppool.tile([NCO, 1], dt, tag="psf")
            nc.tensor.matmul(psf[:], wfc_sb[:], msum[:], start=True, stop=True)
            colv = wpool.tile([NCO, 1], dt, tag="colv")
            nc.vector.tensor_add(colv[:], psf[:], bfc_sb[:])
            # broadcast along the batch shard: bc[p, :] = colv[p]
            bc = wpool.tile([NCO, b_shard], dt, tag="bc")
            nc.vector.tensor_scalar(
                bc[:], zsrc[:], 0.0, colv[:, 0:1], Alu.mult, Alu.add
            )
            nc.sync.dma_start(out_d[:], bc[:])

    nc.compile()
    return nc


def _run_fast(T, b_shard, in_map, trace=False):
    import os

    # The Bass execute path needs the axon jax platform; a caller-pinned
    # JAX_PLATFORMS=cpu (common for running the jax reference) would break it.
    if os.environ.get("JAX_PLATFORMS", "") == "cpu":
        import sys

        if "jax" not in sys.modules:
            del os.environ["JAX_PLATFORMS"]

    from concourse.bass_utils import run_bass_kernel_spmd

    key = (T, b_shard)
    nc = _prog_cache.get(key)
    if nc is None:
        nc = _build_fast_program(T, b_shard)
        _prog_cache[key] = nc
    in_maps = [dict(in_map) for _ in range(N_CORES)]
    return run_bass_kernel_spmd(
        nc, in_maps, list(range(N_CORES)), trace=trace
    )


def _prep_fast_inputs(inputs, T):
    Whh2 = np.asarray(inputs["Whh2"], np.float32)
    b2 = np.asarray(inputs["bih2"], np.float32) + np.asarray(
        inputs["bhh2"], np.float32
    )
    Wfc = np.asarray(inputs["Wfc"], np.float32)
    bfc = np.asarray(inputs["bfc"], np.float32)
    w_np = np.stack(
        [s * Whh2[o : o + H, :].T for o, s in zip(_GATE_OFFS, _GATE_SCALES)],
        axis=1,
    ).reshape(H, 4 * H)
    b_np = np.stack(
        [s * b2[o : o + H] for o, s in zip(_GATE_OFFS, _GATE_SCALES)], axis=1
    )
    # packed fp32 constants: [:,0:NCO]=(Wfc/T).T, [0:NCO,NCO]=bfc,
    # [0:4,8:8+H]=bT (pre-scaled bias, gate-major), [0:4,8+H:12+H]=I4
    p = np.zeros((H, 12 + H), np.float32)
    p[:, 0:NCO] = (Wfc / T).T
    p[0:NCO, NCO] = bfc
    p[0:4, 8 : 8 + H] = b_np.T
    p[0:4, 8 + H : 12 + H] = np.eye(4, dtype=np.float32)
    return {
        "w": np.ascontiguousarray(w_np, np.float16),
        "p": p,
    }


def _sigmoid(x):
    return 1.0 / (1.0 + np.exp(-x))


def _layer2_cpu(inputs, T, B, thr2):
    """Exact fp32 CPU path for thr1 >= 1 but thr2 < 1: layer-2 input is
    still provably zero, so run the batch-1 layer-2 recurrence (with its
    reset logic) on the host and broadcast.  Full precision matters here
    because reset decisions can sit arbitrarily close to the threshold."""
    Whh2 = np.asarray(inputs["Whh2"], np.float32)
    b2 = np.asarray(inputs["bih2"], np.float32) + np.asarray(
        inputs["bhh2"], np.float32
    )
    Wfc = np.asarray(inputs["Wfc"], np.float32)
    bfc = np.asarray(inputs["bfc"], np.float32)
    thr2 = np.float32(thr2)
    syn = np.zeros(H, np.float32)
    mem = np.zeros(H, np.float32)
    msum = np.zeros(H, np.float32)
    for _t in range(T):
        reset = (mem > thr2).astype(np.float32)
        g = mem @ Whh2.T.astype(np.float32) + b2
        i, f, gg, o = np.split(g.astype(np.float32), 4)
        syn = _sigmoid(f) * syn + _sigmoid(i) * np.tanh(gg)
        mem = _sigmoid(o) * np.tanh(syn) - reset * thr2
        msum = msum + mem
    row = (msum / np.float32(T)) @ Wfc.T.astype(np.float32) + bfc
    return np.ascontiguousarray(
        np.broadcast_to(row.astype(np.float32), (B, NCO)), np.float32
    )


def _full_cpu_fallback(inputs):
    """Bit-faithful CPU implementation of the full 2-layer SLSTM reference.
    Only reachable when thr1 < 1.0 (layer-1 spikes possible), which never
    happens for this problem's inputs."""
    x = np.asarray(inputs["x"], np.float32)
    T, B, _C = x.shape
    thr1 = np.float32(np.asarray(inputs["thr1"]))
    thr2 = np.float32(np.asarray(inputs["thr2"]))
    Wih1 = np.asarray(inputs["Wih1"], np.float32)
    Whh1 = np.asarray(inputs["Whh1"], np.float32)
    b1 = np.asarray(inputs["bih1"], np.float32) + np.asarray(
        inputs["bhh1"], np.float32
    )
    Wih2 = np.asarray(inputs["Wih2"], np.float32)
    Whh2 = np.asarray(inputs["Whh2"], np.float32)
    b2 = np.asarray(inputs["bih2"], np.float32) + np.asarray(
        inputs["bhh2"], np.float32
    )
    Wfc = np.asarray(inputs["Wfc"], np.float32)
    bfc = np.asarray(inputs["bfc"], np.float32)

    def cell(xt, mem, syn, Wih, Whh, b):
        g = xt @ Wih.T + mem @ Whh.T + b
        i, f, gg, o = np.split(g, 4, axis=-1)
        c2 = _sigmoid(f) * syn + _sigmoid(i) * np.tanh(gg)
        h = _sigmoid(o) * np.tanh(c2)
        return h, c2

    z = np.zeros((B, H), np.float32)
    syn1, mem1, syn2, mem2 = z.copy(), z.copy(), z.copy(), z.copy()
    msum = np.zeros((B, H), np.float32)
    for t in range(T):
        reset1 = (mem1 > thr1).astype(np.float32)
        h1, syn1 = cell(x[t], mem1, syn1, Wih1, Whh1, b1)
        mem1 = h1 - reset1 * thr1
        spk1 = (mem1 > thr1).astype(np.float32)
        reset2 = (mem2 > thr2).astype(np.float32)
        h2, syn2 = cell(spk1, mem2, syn2, Wih2, Whh2, b2)
        mem2 = h2 - reset2 * thr2
        msum += mem2
    final = msum / np.float32(T)
    return (final @ Wfc.T + bfc).astype(np.float32)


def kernel(**inputs) -> np.ndarray:
    x = np.asarray(inputs["x"])
    T, B = int(x.shape[0]), int(x.shape[1])
    thr1 = float(np.asarray(inputs["thr1"]))
    thr2 = float(np.asarray(inputs["thr2"]))

    # Guard for the fast paths: thr1 >= 1.0 provably kills every layer-1
    # spike (see module docstring), making the output x- and batch-independent.
    shapes_ok = (
        np.asarray(inputs["Whh2"]).shape == (4 * H, H)
        and np.asarray(inputs["Wfc"]).shape == (NCO, H)
        and B % N_CORES == 0
        and B >= N_CORES
        and T >= 1
    )
    if not (thr1 >= 1.0) or not shapes_ok:
        return _full_cpu_fallback(inputs)

    # thr2 >= 1 (or NaN): layer-2 resets are provably zero too -> HW kernel.
    # thr2 < 1: resets can fire with hair-thin margins; use the exact fp32
    # CPU layer-2 path instead (never the case for this problem's inputs).
    if thr2 < 1.0:
        return _layer2_cpu(inputs, T, B, thr2)

    b_shard = B // N_CORES
    in_map = _prep_fast_inputs(inputs, T)
    try:
        res = _run_fast(T, b_shard, in_map, trace=False)
    except Exception:
        # device stack unavailable (e.g. caller pinned jax to cpu before
        # importing us) — fall back to the mathematically equivalent exact
        # CPU path rather than fail.
        return _layer2_cpu(inputs, T, B, thr2)
    out = np.concatenate([r["out"].T for r in res.results], axis=0)
    return np.ascontiguousarray(out, np.float32)



# revision 3
# speedup vs baseline: 1.5503x; 1.5503x over previous
"""Trainium2 Bass kernel for the stacked spiking-LSTM (SLSTM) network.

Problem: x[T=100, B=4096, C=14] -> two snntorch-style SLSTM layers (H=128,
reset_mechanism='subtract', threshold inputs thr1/thr2) -> mean over time of
layer-2 membrane potential -> linear head [B, 7].

Key mathematical property (exploited by the fast path, with a runtime guard):
the spike nonlinearity fires iff mem > thr, and mem = sigmoid(o)*tanh(c)
- reset*thr where |sigmoid(o)*tanh(c)| <= 1 in exact *and* fp32 arithmetic
(both factors saturate at 1.0; a product of two numbers <= 1 rounds to <= 1).
Hence whenever thr1 >= 1.0, layer 1 can never emit a spike, for ANY x and any
weights (even NaN/Inf inputs: NaN > thr is False).  Layer 2 then receives
identically-zero input, so its recurrence is independent of both x and the
batch index, and every output row equals

    out_row = (1/T * sum_t mem2_t) @ Wfc.T + bfc

where mem2_t follows the zero-input LSTM recurrence.  When additionally
thr2 >= 1.0 (the benchmark case) the same saturation argument kills layer-2's
resets, and the HW fast path computes the recurrence on the NeuronCores
(batch-1 column layout, one fused tanh per step via sigmoid(x) =
0.5*tanh(x/2)+0.5 with weight pre-scaling, fp16 matmul operands for Fast
Weight Load, fp32 everywhere else) and broadcasts the row on-device into each
core's batch shard.  thr2 < 1.0 falls back to an exact fp32 CPU layer-2 path
(reset decisions can be margin-critical there); thr1 < 1.0 falls back to a
full-fidelity CPU implementation.  Neither fallback triggers for this
problem's inputs.

Sharding: pure data parallel over batch, B/8 = 512 rows per core; each core
produces its own [7, 512] output shard (identical contents by the theorem
above), gathered and transposed on the host into [B, 7].
"""

import numpy as np

H = 128          # hidden size
NCO = 7          # number of classes
N_CORES = 8

# gate columns [i, f, o, g]; torch gate order in the 4H dim is i,f,g,o
_GATE_OFFS = (0, H, 3 * H, 2 * H)
_GATE_SCALES = (0.5, 0.5, 0.5, 1.0)   # 0.5 for sigmoid-via-tanh gates

_prog_cache: dict = {}


def _build_fast_program(T: int, b_shard: int):
    """Bass/Tile program: zero-input layer-2 SLSTM recurrence at batch 1,
    time-mean, linear head, and on-device broadcast to [NCO, b_shard].

    The recurrent matmul operands (Whh2 chunks and the mem state) are fp16 —
    this engages the PE Fast Weight Load path (4x faster LDWEIGHTS, which sits
    on the serial critical chain) and costs only ~3e-5 relative error on the
    final output (measured; weights are ~1e-1 scale so fp16 keeps ~11 bits).
    Everything else (bias, syn/msum accumulation, activations, head) is fp32.

    Only built for the reset-free regime (thr2 >= 1.0, or thr2 NaN): there
    the layer-2 reset is provably always zero (same saturation argument as
    layer 1: |sigmoid*tanh| <= 1, and NaN > thr is False), so the reset terms
    are dropped from the program entirely and thr2 never enters it."""
    import concourse.bass as bass
    import concourse.bacc as bacc
    import concourse.tile as tile
    import concourse.mybir as mybir

    dt = mybir.dt.float32
    dth = mybir.dt.float16
    Alu = mybir.AluOpType
    Act = mybir.ActivationFunctionType

    # Bacc (not raw Bass): its compile() runs generate_event_semaphores,
    # which splits multi-semaphore waits down to the HW's 1-wait/instruction.
    nc = bacc.Bacc(
        "TRN2", target_bir_lowering=False, debug=False, num_devices=N_CORES
    )
    # fp32 constants travel in one packed tensor (single DMA):
    # [:, 0:NCO]=wfc, [0:NCO, NCO]=bfc, [0:4, 8:8+H]=bT, [0:4, 8+H:12+H]=id4
    PW = 12 + H
    w_d = nc.dram_tensor("w", [H, 4 * H], dth, kind="ExternalInput")
    p_d = nc.dram_tensor("p", [H, PW], dt, kind="ExternalInput")
    out_d = nc.dram_tensor("out", [NCO, b_shard], dt, kind="ExternalOutput")

    with tile.TileContext(nc) as tc:
        with (
            tc.tile_pool(name="const", bufs=1) as cpool,
            tc.tile_pool(name="state", bufs=1) as spool,
            tc.tile_pool(name="work", bufs=4) as wpool,
            tc.tile_pool(name="psum", bufs=4, space=bass.MemorySpace.PSUM) as ppool,
        ):
            # Each DMA fans out over up to 8 HW queue semaphores, and compute
            # instructions have a small sync-wait budget — so every input is
            # staged through a DVE copy whose *only* dependency is its own
            # DMA; all downstream consumers then wait on the DVE semaphore.
            w_stage = cpool.tile([H, 4 * H], dth, tag="w_stage")
            p_stage = cpool.tile([H, PW], dt, tag="p_stage")
            w_sb = cpool.tile([H, 4 * H], dth, tag="w")
            p_sb = cpool.tile([H, PW], dt, tag="p")
            zsrc = cpool.tile([NCO, b_shard], dt, tag="zsrc")
            nc.sync.dma_start(w_stage[:], w_d[:])
            nc.sync.dma_start(p_stage[:], p_d[:])
            nc.vector.tensor_copy(w_sb[:], w_stage[:])
            nc.vector.tensor_copy(p_sb[:], p_stage[:])
            nc.vector.memset(zsrc[:], 0.0)
            wfc_sb = p_sb[:, 0:NCO]
            bfc_sb = p_sb[0:NCO, NCO : NCO + 1]
            bt_sb = p_sb[0:4, 8 : 8 + H]
            id4_sb = p_sb[0:4, 8 + H : 12 + H]

            syn = spool.tile([H, 1], dt, tag="syn")
            msum = spool.tile([H, 1], dt, tag="msum")
            nc.vector.memset(syn[:], 0.0)
            nc.vector.memset(msum[:], 0.0)
            mem_h = wpool.tile([H, 1], dth, tag="memh")
            nc.vector.memset(mem_h[:], 0.0)

            # State convention: mem_h holds 2*mem ("halfmem" = (t_o+1)*tanh(c)),
            # with the compensating 0.5 folded into the recurrent weights (all
            # four gate chunks) and into wfc.  Every tensor op below has
            # free-size-1 operands ([H,1] columns), which keeps each op off the
            # engines' SBUF-access-latency path (an [H,4] op pays a ~185ns
            # non-pipelineable access penalty; [H,1] ops pay none), and the
            # per-gate tanhs replace the sigmoid via sigmoid(x)=0.5*tanh(x/2)+0.5
            # with the 0.5s pre-folded (weights) or post-folded (syn combine).
            for _t in range(T):
                # gates (pre-scaled): ps[:, c] = b_c + w_c.T @ mem.
                # The bias lands in PSUM via a K=4 identity matmul
                # (ps[j,c] = sum_k bT[k,j]*I[k,c]) that has no dependency on
                # mem, so it runs off the critical chain.
                ps = ppool.tile([H, 4], dt, tag="ps")
                nc.tensor.matmul(
                    ps[:], bt_sb[:], id4_sb[:], start=True, stop=False,
                    skip_group_check=True,
                )
                for c in range(4):
                    nc.tensor.matmul(
                        ps[:, c : c + 1],
                        w_sb[:, c * H : (c + 1) * H],
                        mem_h[:],
                        start=False,
                        stop=(c == 3),
                        skip_group_check=True,
                    )
                # four scalar tanhs, one per gate column (i, f, g, o)
                ti = wpool.tile([H, 1], dt, tag="ti")
                nc.scalar.activation(ti[:], ps[:, 0:1], Act.Tanh)
                tf = wpool.tile([H, 1], dt, tag="tf")
                nc.scalar.activation(tf[:], ps[:, 1:2], Act.Tanh)
                tg = wpool.tile([H, 1], dt, tag="tg")
                nc.scalar.activation(tg[:], ps[:, 3:4], Act.Tanh)
                to = wpool.tile([H, 1], dt, tag="to")
                nc.scalar.activation(to[:], ps[:, 2:3], Act.Tanh)
                # syn' = sigmoid(f)*syn + sigmoid(i)*tanh(g)
                #      = 0.5*((t_f+1)*syn + (t_i+1)*t_g)
                zt = wpool.tile([H, 1], dt, tag="z2x")
                nc.vector.scalar_tensor_tensor(
                    zt[:], tf[:], 1.0, syn[:], Alu.add, Alu.mult
                )
                wt = wpool.tile([H, 1], dt, tag="w2x")
                nc.vector.scalar_tensor_tensor(
                    wt[:], ti[:], 1.0, tg[:], Alu.add, Alu.mult
                )
                nc.vector.tensor_scalar(
                    syn[:], zt[:], wt[:, 0:1], 0.5, Alu.add, Alu.mult
                )
                tc2 = wpool.tile([H, 1], dt, tag="tc2")
                nc.scalar.activation(tc2[:], syn[:], Act.Tanh)
                # halfmem = (t_o+1)*tanh(syn) = 2*sigmoid(o)*tanh(syn); the
                # fp16 output feeds the next step's matmuls (Fast Weight Load
                # moving operand) and the fp32 time-sum.
                mem_h = wpool.tile([H, 1], dth, tag="memh")
                nc.vector.scalar_tensor_tensor(
                    mem_h[:], to[:], 1.0, tc2[:], Alu.add, Alu.mult
                )
                nc.vector.tensor_add(msum[:], msum[:], mem_h[:])

            # head: out_col = (Wfc/T).T.T @ msum + bfc  (1/T folded into wfc)
            psf = ppool.tile([NCO, 1], dt, tag="psf")
            nc.tensor.matmul(psf[:], wfc_sb[:], msum[:], start=True, stop=True)
            colv = wpool.tile([NCO, 1], dt, tag="colv")
            nc.vector.tensor_add(colv[:], psf[:], bfc_sb[:])
            # broadcast along the batch shard: bc[p, :] = colv[p]
            bc = wpool.tile([NCO, b_shard], dt, tag="bc")
            nc.vector.tensor_scalar(
                bc[:], zsrc[:], 0.0, colv[:, 0:1], Alu.mult, Alu.add
            )
            nc.sync.dma_start(out_d[:], bc[:])

    nc.compile()
    return nc


def _run_fast(T, b_shard, in_map, trace=False):
    import os

    # The Bass execute path needs the axon jax platform; a caller-pinned
    # JAX_PLATFORMS=cpu (common for running the jax reference) would break it.
    if os.environ.get("JAX_PLATFORMS", "") == "cpu":
        import sys

        if "jax" not in sys.modules:
            del os.environ["JAX_PLATFORMS"]

    from concourse.bass_utils import run_bass_kernel_spmd

    key = (T, b_shard)
    nc = _prog_cache.get(key)
    if nc is None:
        nc = _build_fast_program(T, b_shard)
        _prog_cache[key] = nc
    in_maps = [dict(in_map) for _ in range(N_CORES)]
    return run_bass_kernel_spmd(
        nc, in_maps, list(range(N_CORES)), trace=trace
    )


def _prep_fast_inputs(inputs, T):
    Whh2 = np.asarray(inputs["Whh2"], np.float32)
    b2 = np.asarray(inputs["bih2"], np.float32) + np.asarray(
        inputs["bhh2"], np.float32
    )
    Wfc = np.asarray(inputs["Wfc"], np.float32)
    bfc = np.asarray(inputs["bfc"], np.float32)
    # Extra 0.5 on every gate chunk: the on-device state is halfmem = 2*mem
    # (see _build_fast_program), so W @ mem = (0.5*W) @ halfmem.
    w_np = np.stack(
        [0.5 * s * Whh2[o : o + H, :].T for o, s in zip(_GATE_OFFS, _GATE_SCALES)],
        axis=1,
    ).reshape(H, 4 * H)
    b_np = np.stack(
        [s * b2[o : o + H] for o, s in zip(_GATE_OFFS, _GATE_SCALES)], axis=1
    )
    # packed fp32 constants: [:,0:NCO]=(Wfc/(2T)).T (msum holds halfmem sums),
    # [0:NCO,NCO]=bfc, [0:4,8:8+H]=bT (pre-scaled bias, gate-major),
    # [0:4,8+H:12+H]=I4
    p = np.zeros((H, 12 + H), np.float32)
    p[:, 0:NCO] = (Wfc / (2 * T)).T
    p[0:NCO, NCO] = bfc
    p[0:4, 8 : 8 + H] = b_np.T
    p[0:4, 8 + H : 12 + H] = np.eye(4, dtype=np.float32)
    return {
        "w": np.ascontiguousarray(w_np, np.float16),
        "p": p,
    }


def _sigmoid(x):
    return 1.0 / (1.0 + np.exp(-x))


def _layer2_cpu(inputs, T, B, thr2):
    """Exact fp32 CPU path for thr1 >= 1 but thr2 < 1: layer-2 input is
    still provably zero, so run the batch-1 layer-2 recurrence (with its
    reset logic) on the host and broadcast.  Full precision matters here
    because reset decisions can sit arbitrarily close to the threshold."""
    Whh2 = np.asarray(inputs["Whh2"], np.float32)
    b2 = np.asarray(inputs["bih2"], np.float32) + np.asarray(
        inputs["bhh2"], np.float32
    )
    Wfc = np.asarray(inputs["Wfc"], np.float32)
    bfc = np.asarray(inputs["bfc"], np.float32)
    thr2 = np.float32(thr2)
    syn = np.zeros(H, np.float32)
    mem = np.zeros(H, np.float32)
    msum = np.zeros(H, np.float32)
    for _t in range(T):
        reset = (mem > thr2).astype(np.float32)
        g = mem @ Whh2.T.astype(np.float32) + b2
        i, f, gg, o = np.split(g.astype(np.float32), 4)
        syn = _sigmoid(f) * syn + _sigmoid(i) * np.tanh(gg)
        mem = _sigmoid(o) * np.tanh(syn) - reset * thr2
        msum = msum + mem
    row = (msum / np.float32(T)) @ Wfc.T.astype(np.float32) + bfc
    return np.ascontiguousarray(
        np.broadcast_to(row.astype(np.float32), (B, NCO)), np.float32
    )


def _full_cpu_fallback(inputs):
    """Bit-faithful CPU implementation of the full 2-layer SLSTM reference.
    Only reachable when thr1 < 1.0 (layer-1 spikes possible), which never
    happens for this problem's inputs."""
    x = np.asarray(inputs["x"], np.float32)
    T, B, _C = x.shape
    thr1 = np.float32(np.asarray(inputs["thr1"]))
    thr2 = np.float32(np.asarray(inputs["thr2"]))
    Wih1 = np.asarray(inputs["Wih1"], np.float32)
    Whh1 = np.asarray(inputs["Whh1"], np.float32)
    b1 = np.asarray(inputs["bih1"], np.float32) + np.asarray(
        inputs["bhh1"], np.float32
    )
    Wih2 = np.asarray(inputs["Wih2"], np.float32)
    Whh2 = np.asarray(inputs["Whh2"], np.float32)
    b2 = np.asarray(inputs["bih2"], np.float32) + np.asarray(
        inputs["bhh2"], np.float32
    )
    Wfc = np.asarray(inputs["Wfc"], np.float32)
    bfc = np.asarray(inputs["bfc"], np.float32)

    def cell(xt, mem, syn, Wih, Whh, b):
        g = xt @ Wih.T + mem @ Whh.T + b
        i, f, gg, o = np.split(g, 4, axis=-1)
        c2 = _sigmoid(f) * syn + _sigmoid(i) * np.tanh(gg)
        h = _sigmoid(o) * np.tanh(c2)
        return h, c2

    z = np.zeros((B, H), np.float32)
    syn1, mem1, syn2, mem2 = z.copy(), z.copy(), z.copy(), z.copy()
    msum = np.zeros((B, H), np.float32)
    for t in range(T):
        reset1 = (mem1 > thr1).astype(np.float32)
        h1, syn1 = cell(x[t], mem1, syn1, Wih1, Whh1, b1)
        mem1 = h1 - reset1 * thr1
        spk1 = (mem1 > thr1).astype(np.float32)
        reset2 = (mem2 > thr2).astype(np.float32)
        h2, syn2 = cell(spk1, mem2, syn2, Wih2, Whh2, b2)
        mem2 = h2 - reset2 * thr2
        msum += mem2
    final = msum / np.float32(T)
    return (final @ Wfc.T + bfc).astype(np.float32)


def kernel(**inputs) -> np.ndarray:
    x = np.asarray(inputs["x"])
    T, B = int(x.shape[0]), int(x.shape[1])
    thr1 = float(np.asarray(inputs["thr1"]))
    thr2 = float(np.asarray(inputs["thr2"]))

    # Guard for the fast paths: thr1 >= 1.0 provably kills every layer-1
    # spike (see module docstring), making the output x- and batch-independent.
    shapes_ok = (
        np.asarray(inputs["Whh2"]).shape == (4 * H, H)
        and np.asarray(inputs["Wfc"]).shape == (NCO, H)
        and B % N_CORES == 0
        and B >= N_CORES
        and T >= 1
    )
    if not (thr1 >= 1.0) or not shapes_ok:
        return _full_cpu_fallback(inputs)

    # thr2 >= 1 (or NaN): layer-2 resets are provably zero too -> HW kernel.
    # thr2 < 1: resets can fire with hair-thin margins; use the exact fp32
    # CPU layer-2 path instead (never the case for this problem's inputs).
    if thr2 < 1.0:
        return _layer2_cpu(inputs, T, B, thr2)

    b_shard = B // N_CORES
    in_map = _prep_fast_inputs(inputs, T)
    try:
        res = _run_fast(T, b_shard, in_map, trace=False)
    except Exception:
        # device stack unavailable (e.g. caller pinned jax to cpu before
        # importing us) — fall back to the mathematically equivalent exact
        # CPU path rather than fail.
        return _layer2_cpu(inputs, T, B, thr2)
    out = np.concatenate([r["out"].T for r in res.results], axis=0)
    return np.ascontiguousarray(out, np.float32)



# revision 6
# speedup vs baseline: 1.7270x; 1.1139x over previous
"""Trainium2 Bass kernel for the stacked spiking-LSTM (SLSTM) network.

Problem: x[T=100, B=4096, C=14] -> two snntorch-style SLSTM layers (H=128,
reset_mechanism='subtract', threshold inputs thr1/thr2) -> mean over time of
layer-2 membrane potential -> linear head [B, 7].

Key mathematical property (exploited by the fast path, with a runtime guard):
the spike nonlinearity fires iff mem > thr, and mem = sigmoid(o)*tanh(c)
- reset*thr where |sigmoid(o)*tanh(c)| <= 1 in exact *and* fp32 arithmetic
(both factors saturate at 1.0; a product of two numbers <= 1 rounds to <= 1).
Hence whenever thr1 >= 1.0, layer 1 can never emit a spike, for ANY x and any
weights (even NaN/Inf inputs: NaN > thr is False).  Layer 2 then receives
identically-zero input, so its recurrence is independent of both x and the
batch index, and every output row equals

    out_row = (1/T * sum_t mem2_t) @ Wfc.T + bfc

where mem2_t follows the zero-input LSTM recurrence.  When additionally
thr2 >= 1.0 (the benchmark case) the same saturation argument kills layer-2's
resets, and the HW fast path computes the recurrence on the NeuronCores
(batch-1 column layout, one fused tanh per step via sigmoid(x) =
0.5*tanh(x/2)+0.5 with weight pre-scaling, fp16 matmul operands for Fast
Weight Load, fp32 everywhere else) and broadcasts the row on-device into each
core's batch shard.  thr2 < 1.0 falls back to an exact fp32 CPU layer-2 path
(reset decisions can be margin-critical there); thr1 < 1.0 falls back to a
full-fidelity CPU implementation.  Neither fallback triggers for this
problem's inputs.

Sharding: pure data parallel over batch, B/8 = 512 rows per core; each core
produces its own [7, 512] output shard (identical contents by the theorem
above), gathered and transposed on the host into [B, 7].
"""

import numpy as np

H = 128          # hidden size
NCO = 7          # number of classes
N_CORES = 8

# gate columns [g, i, f, o]; torch gate order in the 4H dim is i,f,g,o.
# g first so the tanh(g) activation (first on the chain) waits on the
# earliest matmul.
_GATE_OFFS = (2 * H, 0, H, 3 * H)

_prog_cache: dict = {}


def _build_fast_program(T: int, b_shard: int):
    """Bass/Tile program: zero-input layer-2 SLSTM recurrence at batch 1,
    time-mean, linear head, and on-device broadcast to [NCO, b_shard].

    The recurrent matmul operands (Whh2 chunks and the mem state) are fp16 —
    this engages the PE Fast Weight Load path (4x faster LDWEIGHTS, which sits
    on the serial critical chain) and costs only ~3e-5 relative error on the
    final output (measured; weights are ~1e-1 scale so fp16 keeps ~11 bits).
    Everything else (bias, syn/msum accumulation, activations, head) is fp32.

    Only built for the reset-free regime (thr2 >= 1.0, or thr2 NaN): there
    the layer-2 reset is provably always zero (same saturation argument as
    layer 1: |sigmoid*tanh| <= 1, and NaN > thr is False), so the reset terms
    are dropped from the program entirely and thr2 never enters it."""
    import concourse.bass as bass
    import concourse.bacc as bacc
    import concourse.tile as tile
    import concourse.mybir as mybir

    dt = mybir.dt.float32
    dth = mybir.dt.float16
    Alu = mybir.AluOpType
    Act = mybir.ActivationFunctionType

    # Bacc (not raw Bass): its compile() runs generate_event_semaphores,
    # which splits multi-semaphore waits down to the HW's 1-wait/instruction.
    nc = bacc.Bacc(
        "TRN2", target_bir_lowering=False, debug=False, num_devices=N_CORES
    )
    # fp32 constants travel in one packed tensor (single DMA):
    # [:, 0:NCO]=wfc, [0:NCO, NCO]=bfc, [0:4, 8:8+H]=bT, [0:4, 8+H:12+H]=id4
    PW = 12 + H
    w_d = nc.dram_tensor("w", [H, 4 * H], dth, kind="ExternalInput")
    p_d = nc.dram_tensor("p", [H, PW], dt, kind="ExternalInput")
    out_d = nc.dram_tensor("out", [NCO, b_shard], dt, kind="ExternalOutput")

    with tile.TileContext(nc) as tc:
        with (
            tc.tile_pool(name="const", bufs=1) as cpool,
            tc.tile_pool(name="state", bufs=1) as spool,
            tc.tile_pool(name="work", bufs=4) as wpool,
            tc.tile_pool(name="psum", bufs=4, space=bass.MemorySpace.PSUM) as ppool,
        ):
            # Each DMA fans out over up to 8 HW queue semaphores, and compute
            # instructions have a small sync-wait budget — so every input is
            # staged through a DVE copy whose *only* dependency is its own
            # DMA; all downstream consumers then wait on the DVE semaphore.
            w_stage = cpool.tile([H, 4 * H], dth, tag="w_stage")
            p_stage = cpool.tile([H, PW], dt, tag="p_stage")
            w_sb = cpool.tile([H, 4 * H], dth, tag="w")
            p_sb = cpool.tile([H, PW], dt, tag="p")
            zsrc = cpool.tile([NCO, b_shard], dt, tag="zsrc")
            nc.sync.dma_start(w_stage[:], w_d[:])
            nc.sync.dma_start(p_stage[:], p_d[:])
            nc.vector.tensor_copy(w_sb[:], w_stage[:])
            nc.vector.tensor_copy(p_sb[:], p_stage[:])
            nc.vector.memset(zsrc[:], 0.0)
            wfc_sb = p_sb[:, 0:NCO]
            bfc_sb = p_sb[0:NCO, NCO : NCO + 1]
            bt_sb = p_sb[0:4, 8 : 8 + H]
            id4_sb = p_sb[0:4, 8 + H : 12 + H]

            syn = spool.tile([H, 1], dt, tag="syn")
            msum = spool.tile([H, 1], dt, tag="msum")
            nc.vector.memset(syn[:], 0.0)
            nc.vector.memset(msum[:], 0.0)
            mem_h = wpool.tile([H, 1], dth, tag="memh")
            nc.vector.memset(mem_h[:], 0.0)

            # Chain-latency-optimized step.  Every tensor op has free-size-1
            # operands ([H,1] columns), which keeps each op off the engines'
            # SBUF-access-latency path (an [H,4] op pays a ~185ns
            # non-pipelineable access penalty; [H,1] ops pay none).  The
            # serial chain per step is PE(matmuls) -> ACT(tg) -> ACT(si) ->
            # DVE(wt=si*tg) -> ACT(tc2=Tanh(sf*syn+wt)) -> ACT(memh=so*tc2)
            # -> PE; sf/so are computed on ACT in the slack while wt makes
            # its DVE round trip, and the syn-state update + time-sum run on
            # DVE entirely off the chain.  Cross-engine semaphore updates
            # hold their producer engine ~34ns each, so ops whose consumers
            # are same-engine-only are kept update-free.
            for _t in range(T):
                # gates: ps[:, c] = b_c + w_c.T @ mem, columns (g, i, f, o).
                # The bias lands in PSUM via a K=4 identity matmul
                # (ps[j,c] = sum_k bT[k,j]*I[k,c]) that has no dependency on
                # mem, so it runs off the critical chain.
                ps = ppool.tile([H, 4], dt, tag="ps")
                nc.tensor.matmul(
                    ps[:], bt_sb[:], id4_sb[:], start=True, stop=False,
                    skip_group_check=True,
                )
                for c in range(4):
                    nc.tensor.matmul(
                        ps[:, c : c + 1],
                        w_sb[:, c * H : (c + 1) * H],
                        mem_h[:],
                        start=False,
                        stop=(c == 3),
                        skip_group_check=True,
                    )
                # per-gate scalar activations: tanh(g), then the three
                # sigmoids.  tg/si feed DVE (wt); sf/so feed only ACT ops.
                tg = wpool.tile([H, 1], dt, tag="tg")
                nc.scalar.activation(tg[:], ps[:, 0:1], Act.Tanh)
                si = wpool.tile([H, 1], dt, tag="si")
                nc.scalar.activation(si[:], ps[:, 1:2], Act.Sigmoid)
                sf = wpool.tile([H, 1], dt, tag="sf")
                nc.scalar.activation(sf[:], ps[:, 2:3], Act.Sigmoid)
                so = wpool.tile([H, 1], dt, tag="so")
                nc.scalar.activation(so[:], ps[:, 3:4], Act.Sigmoid)
                # wt = sigmoid(i)*tanh(g) on DVE (the only chain op off ACT)
                wt = wpool.tile([H, 1], dt, tag="wt")
                nc.vector.tensor_mul(wt[:], si[:], tg[:])
                # tc2 = tanh(sigmoid(f)*syn + wt) — scale/bias-fused tanh
                tc2 = wpool.tile([H, 1], dt, tag="tc2")
                nc.scalar.activation(
                    tc2[:], syn[:], Act.Tanh, bias=wt[:, 0:1], scale=sf[:, 0:1]
                )
                # mem = sigmoid(o)*tanh(syn') in fp16 for next step's matmuls
                mem_h = wpool.tile([H, 1], dth, tag="memh")
                nc.scalar.activation(mem_h[:], tc2[:], Act.Identity, scale=so[:, 0:1])
                # off-chain on DVE: syn state update + time-sum
                syn_new = wpool.tile([H, 1], dt, tag="syn")
                nc.vector.scalar_tensor_tensor(
                    syn_new[:], syn[:], sf[:, 0:1], wt[:], Alu.mult, Alu.add
                )
                syn = syn_new
                nc.vector.tensor_add(msum[:], msum[:], mem_h[:])

            # head: out_col = (Wfc/T).T.T @ msum + bfc  (1/T folded into wfc)
            psf = ppool.tile([NCO, 1], dt, tag="psf")
            nc.tensor.matmul(psf[:], wfc_sb[:], msum[:], start=True, stop=True)
            colv = wpool.tile([NCO, 1], dt, tag="colv")
            nc.vector.tensor_add(colv[:], psf[:], bfc_sb[:])
            # broadcast along the batch shard: bc[p, :] = colv[p]
            bc = wpool.tile([NCO, b_shard], dt, tag="bc")
            nc.vector.tensor_scalar(
                bc[:], zsrc[:], 0.0, colv[:, 0:1], Alu.mult, Alu.add
            )
            nc.sync.dma_start(out_d[:], bc[:])

    nc.compile()
    return nc


def _run_fast(T, b_shard, in_map, trace=False):
    import os

    # The Bass execute path needs the axon jax platform; a caller-pinned
    # JAX_PLATFORMS=cpu (common for running the jax reference) would break it.
    if os.environ.get("JAX_PLATFORMS", "") == "cpu":
        import sys

        if "jax" not in sys.modules:
            del os.environ["JAX_PLATFORMS"]

    from concourse.bass_utils import run_bass_kernel_spmd

    key = (T, b_shard)
    nc = _prog_cache.get(key)
    if nc is None:
        nc = _build_fast_program(T, b_shard)
        _prog_cache[key] = nc
    in_maps = [dict(in_map) for _ in range(N_CORES)]
    return run_bass_kernel_spmd(
        nc, in_maps, list(range(N_CORES)), trace=trace
    )


def _prep_fast_inputs(inputs, T):
    Whh2 = np.asarray(inputs["Whh2"], np.float32)
    b2 = np.asarray(inputs["bih2"], np.float32) + np.asarray(
        inputs["bhh2"], np.float32
    )
    Wfc = np.asarray(inputs["Wfc"], np.float32)
    bfc = np.asarray(inputs["bfc"], np.float32)
    # Gate chunks in on-device column order (g, i, f, o), unscaled: the
    # device applies Sigmoid/Tanh directly to the raw pre-activations.
    w_np = np.stack(
        [Whh2[o : o + H, :].T for o in _GATE_OFFS], axis=1
    ).reshape(H, 4 * H)
    b_np = np.stack([b2[o : o + H] for o in _GATE_OFFS], axis=1)
    # packed fp32 constants: [:,0:NCO]=(Wfc/T).T, [0:NCO,NCO]=bfc,
    # [0:4,8:8+H]=bT (bias, gate-major), [0:4,8+H:12+H]=I4
    p = np.zeros((H, 12 + H), np.float32)
    p[:, 0:NCO] = (Wfc / T).T
    p[0:NCO, NCO] = bfc
    p[0:4, 8 : 8 + H] = b_np.T
    p[0:4, 8 + H : 12 + H] = np.eye(4, dtype=np.float32)
    return {
        "w": np.ascontiguousarray(w_np, np.float16),
        "p": p,
    }


def _sigmoid(x):
    return 1.0 / (1.0 + np.exp(-x))


def _layer2_cpu(inputs, T, B, thr2):
    """Exact fp32 CPU path for thr1 >= 1 but thr2 < 1: layer-2 input is
    still provably zero, so run the batch-1 layer-2 recurrence (with its
    reset logic) on the host and broadcast.  Full precision matters here
    because reset decisions can sit arbitrarily close to the threshold."""
    Whh2 = np.asarray(inputs["Whh2"], np.float32)
    b2 = np.asarray(inputs["bih2"], np.float32) + np.asarray(
        inputs["bhh2"], np.float32
    )
    Wfc = np.asarray(inputs["Wfc"], np.float32)
    bfc = np.asarray(inputs["bfc"], np.float32)
    thr2 = np.float32(thr2)
    syn = np.zeros(H, np.float32)
    mem = np.zeros(H, np.float32)
    msum = np.zeros(H, np.float32)
    for _t in range(T):
        reset = (mem > thr2).astype(np.float32)
        g = mem @ Whh2.T.astype(np.float32) + b2
        i, f, gg, o = np.split(g.astype(np.float32), 4)
        syn = _sigmoid(f) * syn + _sigmoid(i) * np.tanh(gg)
        mem = _sigmoid(o) * np.tanh(syn) - reset * thr2
        msum = msum + mem
    row = (msum / np.float32(T)) @ Wfc.T.astype(np.float32) + bfc
    return np.ascontiguousarray(
        np.broadcast_to(row.astype(np.float32), (B, NCO)), np.float32
    )


def _full_cpu_fallback(inputs):
    """Bit-faithful CPU implementation of the full 2-layer SLSTM reference.
    Only reachable when thr1 < 1.0 (layer-1 spikes possible), which never
    happens for this problem's inputs."""
    x = np.asarray(inputs["x"], np.float32)
    T, B, _C = x.shape
    thr1 = np.float32(np.asarray(inputs["thr1"]))
    thr2 = np.float32(np.asarray(inputs["thr2"]))
    Wih1 = np.asarray(inputs["Wih1"], np.float32)
    Whh1 = np.asarray(inputs["Whh1"], np.float32)
    b1 = np.asarray(inputs["bih1"], np.float32) + np.asarray(
        inputs["bhh1"], np.float32
    )
    Wih2 = np.asarray(inputs["Wih2"], np.float32)
    Whh2 = np.asarray(inputs["Whh2"], np.float32)
    b2 = np.asarray(inputs["bih2"], np.float32) + np.asarray(
        inputs["bhh2"], np.float32
    )
    Wfc = np.asarray(inputs["Wfc"], np.float32)
    bfc = np.asarray(inputs["bfc"], np.float32)

    def cell(xt, mem, syn, Wih, Whh, b):
        g = xt @ Wih.T + mem @ Whh.T + b
        i, f, gg, o = np.split(g, 4, axis=-1)
        c2 = _sigmoid(f) * syn + _sigmoid(i) * np.tanh(gg)
        h = _sigmoid(o) * np.tanh(c2)
        return h, c2

    z = np.zeros((B, H), np.float32)
    syn1, mem1, syn2, mem2 = z.copy(), z.copy(), z.copy(), z.copy()
    msum = np.zeros((B, H), np.float32)
    for t in range(T):
        reset1 = (mem1 > thr1).astype(np.float32)
        h1, syn1 = cell(x[t], mem1, syn1, Wih1, Whh1, b1)
        mem1 = h1 - reset1 * thr1
        spk1 = (mem1 > thr1).astype(np.float32)
        reset2 = (mem2 > thr2).astype(np.float32)
        h2, syn2 = cell(spk1, mem2, syn2, Wih2, Whh2, b2)
        mem2 = h2 - reset2 * thr2
        msum += mem2
    final = msum / np.float32(T)
    return (final @ Wfc.T + bfc).astype(np.float32)


def kernel(**inputs) -> np.ndarray:
    x = np.asarray(inputs["x"])
    T, B = int(x.shape[0]), int(x.shape[1])
    thr1 = float(np.asarray(inputs["thr1"]))
    thr2 = float(np.asarray(inputs["thr2"]))

    # Guard for the fast paths: thr1 >= 1.0 provably kills every layer-1
    # spike (see module docstring), making the output x- and batch-independent.
    shapes_ok = (
        np.asarray(inputs["Whh2"]).shape == (4 * H, H)
        and np.asarray(inputs["Wfc"]).shape == (NCO, H)
        and B % N_CORES == 0
        and B >= N_CORES
        and T >= 1
    )
    if not (thr1 >= 1.0) or not shapes_ok:
        return _full_cpu_fallback(inputs)

    # thr2 >= 1 (or NaN): layer-2 resets are provably zero too -> HW kernel.
    # thr2 < 1: resets can fire with hair-thin margins; use the exact fp32
    # CPU layer-2 path instead (never the case for this problem's inputs).
    if thr2 < 1.0:
        return _layer2_cpu(inputs, T, B, thr2)

    b_shard = B // N_CORES
    in_map = _prep_fast_inputs(inputs, T)
    try:
        res = _run_fast(T, b_shard, in_map, trace=False)
    except Exception:
        # device stack unavailable (e.g. caller pinned jax to cpu before
        # importing us) — fall back to the mathematically equivalent exact
        # CPU path rather than fail.
        return _layer2_cpu(inputs, T, B, thr2)
    out = np.concatenate([r["out"].T for r in res.results], axis=0)
    return np.ascontiguousarray(out, np.float32)

